# revision 7
# baseline (speedup 1.0000x reference)
"""Trainium2 Bass kernel for a 2-layer GCN encoder (N=100000, E=1600000, 128->128->64).

v2 strategy (8 NeuronCores, SPMD):
  out = A_hat @ relu(A_hat @ X @ W1 + b1) @ W2 + b2,  A_hat = D^-1/2 (A+I) D^-1/2

  Nodes are degree-sorted and dealt into 784 blocks of 128 (block g ->
  core g%8, local index g//8), so the 8 cores' bl-th blocks have nearly
  identical degree profiles and one static program serves all cores with
  per-block chunk counts.

  Layer 1 (zero one-hots): the per-edge source rows of x are host-gathered,
  norm-folded ((dinv_u*dinv_v) folded in, self-loops included) and stored
  FEAT-MAJOR bf16 in identity-routing layout: chunk j, lane d = j-th edge
  of the dest in slot d (zero rows pad). Then t1[hid,dest] accumulates
  with ONE weight-stationary matmul per chunk: t1 += W1^T @ chunkT.
  Tail: h1 = relu(t1+b1) (ACT, bf16), h2p[dest,o] = h1-lhsT @ W2,
  h2s = h2p * dinv_dest (ACT per-partition scale), duplicated to 128 bf16
  cols so the L2 gather element is 256B.

  AllGather h2_local bf16 [12544,128] -> h2_full [100352,128].

  Layer 2: self-loop contributions via contiguous dma_start from the
  core's OWN h2_local rows + identity matmul (no gather descriptors).
  Non-self edges: SWDGE dma_gather (int16 idx, 4 position buckets) of
  256B bf16 rows, routed lane->dest by bf16 is_eq one-hots (160ns vs
  785ns for the baseline's f32 eq+mult): acc[o,dest] += g[:, :64]^T @ st.
  Tail: DVE mult by dinv_dest tile + add b2; output written transposed
  [64, rows]; host un-transposes and un-permutes.
"""

import math

import numpy as np
import ml_dtypes

N = 100000
E = 1600000
IN_F = 128
HID = 128
OUT_F = 64
NCORES = 8
P = 128
BLOCKS_PER_CORE = 98
NBLOCKS = NCORES * BLOCKS_PER_CORE  # 784
ROWS_PER_CORE = BLOCKS_PER_CORE * P  # 12544
NBUCKET = 4
L2_BUCKET_ROWS = 25088
L1_GROUP_CH = 72   # max chunks per L1 stream group
L2_GROUP_CH = 24   # max chunks per (bucket-call) in an L2 group

_BUILD_CACHE = {}

bf16 = ml_dtypes.bfloat16


def _ranks(key, ncells):
    order = np.argsort(key, kind="stable")
    key_sorted = key[order]
    counts = np.bincount(key_sorted, minlength=ncells)
    starts = np.zeros_like(counts)
    starts[1:] = np.cumsum(counts)[:-1]
    rank_sorted = np.arange(order.size, dtype=np.int64) - starts[key_sorted]
    rank = np.empty(order.size, dtype=np.int64)
    rank[order] = rank_sorted
    return rank, counts


def _pack_gidx(idx_stream):
    m = idx_stream.reshape(-1, 16).T
    return np.ascontiguousarray(np.tile(m, (8, 1)))


def _l1_groups(p1_list):
    """Group consecutive blocks while total chunks <= L1_GROUP_CH."""
    out = []
    b0 = 0
    while b0 < BLOCKS_PER_CORE:
        nb = 0
        tot = 0
        while b0 + nb < BLOCKS_PER_CORE and (
            nb == 0 or tot + p1_list[b0 + nb] <= L1_GROUP_CH
        ):
            tot += p1_list[b0 + nb]
            nb += 1
        out.append((b0, nb))
        b0 += nb
    return out


def _l2_groups(cap2):
    """Group consecutive blocks while per-bucket chunk total <= L2_GROUP_CH."""
    out = []
    b0 = 0
    while b0 < BLOCKS_PER_CORE:
        nb = 0
        while b0 + nb < BLOCKS_PER_CORE and nb < 4:
            cand = [
                sum(cap2[b0 + i][k] for i in range(nb + 1))
                for k in range(NBUCKET)
            ]
            if nb > 0 and max(cand) > L2_GROUP_CH:
                break
            nb += 1
        out.append((b0, nb))
        b0 += nb
    return out


def _prep(x, edge_index, W1, b1, W2, b2):
    x = np.ascontiguousarray(np.asarray(x, dtype=np.float32))
    ei = np.asarray(edge_index, dtype=np.int64)
    loops = np.arange(N, dtype=np.int64)
    row1 = np.concatenate([ei[0], loops])   # L1 includes self loops
    col1 = np.concatenate([ei[1], loops])

    degi = np.bincount(col1, minlength=N)   # includes self loop
    dinv = (1.0 / np.sqrt(degi.astype(np.float64))).astype(np.float32)
    norm1 = dinv[row1] * dinv[col1]

    # degree-sorted deal: sorted rank r -> global block r//P -> core g%8, bl g//8
    order = np.argsort(-degi, kind="stable")
    gblock = np.empty(N, np.int64)
    slot_of = np.empty(N, np.int64)
    r = np.arange(N, dtype=np.int64)
    gblock[order] = r // P
    slot_of[order] = r % P
    core_of = gblock % NCORES
    bl_of = gblock // NCORES
    perm_pos = core_of * ROWS_PER_CORE + bl_of * P + slot_of

    deg_sorted = degi[order]
    # p1_list[bl] = max degree among the 8 cores' bl-th blocks
    p1_list = [int(deg_sorted[bl * NCORES * P]) for bl in range(BLOCKS_PER_CORE)]
    l1_off = np.zeros(BLOCKS_PER_CORE + 1, np.int64)
    l1_off[1:] = np.cumsum(p1_list)
    tot1 = int(l1_off[-1]) * P  # per-core L1 edge slots

    # L1 positions: per core, block bl region at l1_off[bl]*P; chunk j lane d
    rankd, _ = _ranks(col1, N)
    c1 = col1
    pos1 = core_of[c1] * tot1 + (l1_off[bl_of[c1]] + rankd) * P + slot_of[c1]
    src1 = np.zeros(NCORES * tot1, np.int64)
    n1 = np.zeros(NCORES * tot1, np.float32)
    src1[pos1] = row1
    n1[pos1] = norm1

    # ---- layer 2 (non-self edges): 4 position buckets, one-hot routing ----
    dcore = core_of[ei[1]]
    dbl = bl_of[ei[1]]
    dloc_all = slot_of[ei[1]].astype(np.float32)
    cpos = perm_pos[ei[0]]
    b2k = cpos // L2_BUCKET_ROWS
    i2 = (cpos - b2k * L2_BUCKET_ROWS).astype(np.int16)
    key2 = (dcore * BLOCKS_PER_CORE + dbl) * NBUCKET + b2k
    rank2, cnt2 = _ranks(key2, NBLOCKS * NBUCKET)
    cnt2m = cnt2.reshape(NCORES, BLOCKS_PER_CORE, NBUCKET)
    cap2 = [
        [
            int(math.ceil(int(cnt2m[:, bl, k].max()) / P)) * P
            for k in range(NBUCKET)
        ]
        for bl in range(BLOCKS_PER_CORE)
    ]  # slots per (bl, bucket)

    groups2 = _l2_groups([[c // P for c in row] for row in cap2])
    # stream order: group g -> bucket k -> block bl -> slots
    cell_start = np.zeros((BLOCKS_PER_CORE, NBUCKET), np.int64)
    off = 0
    for b0, nb in groups2:
        for k in range(NBUCKET):
            for bl in range(nb):
                cell_start[b0 + bl, k] = off
                off += cap2[b0 + bl][k]
    tot2 = off  # per-core L2 edge slots

    pos2 = dcore * tot2 + cell_start[dbl, b2k] + rank2
    i2s = np.zeros(NCORES * tot2, np.int16)
    i2s[pos2] = i2
    ohfull = np.zeros((NCORES * tot2, P), bf16)
    ohfull[pos2, slot_of[ei[1]]] = bf16(1.0)

    dinv_posarr = np.zeros(NBLOCKS * P, np.float32)
    dinv_posarr[perm_pos] = dinv

    per_core = []
    for s in range(NCORES):
        sl1 = slice(s * tot1, (s + 1) * tot1)
        sl2 = slice(s * tot2, (s + 1) * tot2)
        xs = x[src1[sl1]] * n1[sl1][:, None]  # [tot1, IN_F] f32
        nch1 = tot1 // P
        xgT = np.ascontiguousarray(
            xs.reshape(nch1, P, IN_F).transpose(2, 0, 1).reshape(IN_F, -1)
        ).astype(bf16)
        dslice = dinv_posarr[s * ROWS_PER_CORE : (s + 1) * ROWS_PER_CORE]
        per_core.append(
            {
                "xgT": xgT,
                "dinvp": np.ascontiguousarray(dslice.reshape(-1, P).T),
                "dinvl": np.ascontiguousarray(np.tile(dslice, (OUT_F, 1))),
                "gidx2": _pack_gidx(i2s[sl2]),
                "oh": np.ascontiguousarray(
                    ohfull[sl2].reshape(-1, P, P).transpose(1, 0, 2).reshape(P, -1)
                ),
            }
        )

    iota_np = np.tile(np.arange(P, dtype=np.float32), (P, 1))
    consts = {
        "w1": np.ascontiguousarray(np.asarray(W1, np.float32)).astype(bf16),
        "w2": np.ascontiguousarray(np.asarray(W2, np.float32)).astype(bf16),
        "b1": np.ascontiguousarray(np.asarray(b1, np.float32).reshape(HID, 1)),
        "b2": np.ascontiguousarray(np.asarray(b2, np.float32).reshape(OUT_F, 1)),
        "iotab": np.ascontiguousarray(iota_np).astype(bf16),
        "identb": np.eye(P, dtype=np.float32).astype(bf16),
    }
    key = (tuple(p1_list), tuple(tuple(r_) for r_ in cap2))
    return key, per_core, consts, perm_pos


def _build(key):
    if key in _BUILD_CACHE:
        return _BUILD_CACHE[key]

    import concourse.bass as bass  # noqa: F401
    import concourse.bacc as bacc
    import concourse.mybir as mybir
    import concourse.tile as tile

    p1_list, cap2 = key
    p1_list = list(p1_list)
    cap2_ch = [[c // P for c in row] for row in cap2]
    f32 = mybir.dt.float32
    bf = mybir.dt.bfloat16
    i16 = mybir.dt.int16
    groups1 = _l1_groups(p1_list)
    groups2 = _l2_groups(cap2_ch)
    l1_off = [0]
    for p in p1_list:
        l1_off.append(l1_off[-1] + p)
    nch1 = l1_off[-1]
    nch2 = sum(
        cap2_ch[b0 + bl][k]
        for b0, nb in groups2
        for k in range(NBUCKET)
        for bl in range(nb)
    )

    nc = bacc.Bacc(
        "TRN2", target_bir_lowering=False, debug=False, num_devices=NCORES
    )
    xgT = nc.dram_tensor("xgT", [P, nch1 * P], bf, kind="ExternalInput")
    w1 = nc.dram_tensor("w1", [IN_F, HID], bf, kind="ExternalInput")
    w2 = nc.dram_tensor("w2", [HID, OUT_F], bf, kind="ExternalInput")
    b1 = nc.dram_tensor("b1", [HID, 1], f32, kind="ExternalInput")
    b2 = nc.dram_tensor("b2", [OUT_F, 1], f32, kind="ExternalInput")
    iotab = nc.dram_tensor("iotab", [P, P], bf, kind="ExternalInput")
    identb = nc.dram_tensor("identb", [P, P], bf, kind="ExternalInput")
    dinvp = nc.dram_tensor("dinvp", [P, BLOCKS_PER_CORE], f32, kind="ExternalInput")
    dinvl = nc.dram_tensor("dinvl", [OUT_F, ROWS_PER_CORE], f32, kind="ExternalInput")
    gidx2 = nc.dram_tensor("gidx2", [P, nch2 * P // 16], i16, kind="ExternalInput")
    oh = nc.dram_tensor("oh", [P, nch2 * P], bf, kind="ExternalInput")
    outT = nc.dram_tensor(
        "outT", [OUT_F, ROWS_PER_CORE], f32, kind="ExternalOutput"
    )

    relu = mybir.ActivationFunctionType.Relu
    copyf = mybir.ActivationFunctionType.Copy
    is_eq = mybir.AluOpType.is_equal
    mult = mybir.AluOpType.mult
    add = mybir.AluOpType.add

    with tile.TileContext(nc) as tc:
        with (
            tc.tile_pool(name="consts", bufs=1) as cp,
            tc.tile_pool(name="gat", bufs=2) as gat,
            tc.tile_pool(name="idxp", bufs=2) as idxp,
            tc.tile_pool(name="dnp", bufs=2) as dnp,
            tc.tile_pool(name="sp", bufs=6) as sp,
            tc.tile_pool(name="blk", bufs=3) as blk,
            tc.tile_pool(name="ps1", bufs=2, space="PSUM") as ps1,
            tc.tile_pool(name="ps2", bufs=2, space="PSUM") as ps2,
            tc.tile_pool(name="psl2", bufs=4, space="PSUM") as psl2,
            tc.tile_pool(name="dram", bufs=1, space="DRAM") as dram,
        ):
            w1t = cp.tile([IN_F, HID], bf)
            w2t = cp.tile([HID, OUT_F], bf)
            b1t = cp.tile([HID, 1], f32)
            b2t = cp.tile([OUT_F, 1], f32)
            iot = cp.tile([P, P], bf)
            idt = cp.tile([P, P], bf)
            dvp = cp.tile([P, BLOCKS_PER_CORE], f32)
            nc.sync.dma_start(w1t[:], w1[:])
            nc.sync.dma_start(w2t[:], w2[:])
            nc.sync.dma_start(b1t[:], b1[:])
            nc.sync.dma_start(b2t[:], b2[:])
            nc.sync.dma_start(iot[:], iotab[:])
            nc.sync.dma_start(idt[:], identb[:])
            nc.sync.dma_start(dvp[:], dinvp[:])

            h2_local = dram.tile([ROWS_PER_CORE, P], bf, tag="h2l")
            h2_full = dram.tile(
                [NCORES * ROWS_PER_CORE, P], bf, tag="h2f", addr_space="Shared"
            )

            # ---------------- Layer 1 ----------------
            for b0, nb in groups1:
                c0 = l1_off[b0]
                C = l1_off[b0 + nb] - c0
                gt = gat.tile([P, C * P], bf, tag="g")
                nc.sync.dma_start(gt[:], xgT[:, c0 * P : (c0 + C) * P])
                for bl in range(nb):
                    bb = b0 + bl
                    p1b = p1_list[bb]
                    cb = l1_off[bb] - c0
                    acc = ps1.tile([HID, P], f32, tag="acc1")
                    for j in range(p1b):
                        c = cb + j
                        nc.tensor.matmul(
                            acc[:],
                            lhsT=w1t[:],
                            rhs=gt[:, c * P : (c + 1) * P],
                            start=(j == 0),
                            stop=(j == p1b - 1),
                        )
                    h1 = blk.tile([HID, P], bf, tag="h1")
                    nc.scalar.activation(h1[:], acc[:], relu, bias=b1t[:, :1])
                    h2p = ps2.tile([P, OUT_F], f32, tag="h2p")
                    nc.tensor.matmul(
                        h2p[:], lhsT=h1[:], rhs=w2t[:], start=True, stop=True
                    )
                    h2s = blk.tile([P, P], bf, tag="h2s")
                    nc.scalar.activation(
                        h2s[:, 0:OUT_F], h2p[:], copyf, scale=dvp[:, bb : bb + 1]
                    )
                    nc.vector.tensor_copy(h2s[:, OUT_F:P], h2s[:, 0:OUT_F])
                    nc.sync.dma_start(h2_local[bb * P : (bb + 1) * P, :], h2s[:])

            # ---------------- AllGather ----------------
            nc.gpsimd.collective_compute(
                "AllGather",
                mybir.AluOpType.bypass,
                replica_groups=[list(range(NCORES))],
                ins=[h2_local.opt()],
                outs=[h2_full.opt()],
            )

            # ---------------- Layer 2 ----------------
            chunk_base = 0
            for b0, nb in groups2:
                kch = [
                    sum(cap2_ch[b0 + bl][k] for bl in range(nb))
                    for k in range(NBUCKET)
                ]
                gts = []
                kbase = []
                cb = chunk_base
                for k in range(NBUCKET):
                    nidx = kch[k] * P
                    gt2 = gat.tile([P, kch[k] * P], bf, tag=f"g2{k}")
                    it = idxp.tile([P, nidx // 16], i16, tag=f"i{k}")
                    nc.sync.dma_start(
                        it[:], gidx2[:, cb * P // 16 : (cb + kch[k]) * P // 16]
                    )
                    nc.gpsimd.dma_gather(
                        out_ap=gt2[:].rearrange("p (c e) -> p c e", e=P),
                        in_ap=h2_full[
                            k * L2_BUCKET_ROWS : (k + 1) * L2_BUCKET_ROWS, :
                        ],
                        idxs_ap=it[:],
                        num_idxs=nidx,
                        num_idxs_reg=nidx,
                        elem_size=P,
                        single_packet=False,
                    )
                    gts.append(gt2)
                    kbase.append(cb)
                    cb += kch[k]
                tot_ch = cb - chunk_base
                oht = dnp.tile([P, tot_ch * P], bf, tag="oh")
                nc.sync.dma_start(
                    oht[:], oh[:, chunk_base * P : (chunk_base + tot_ch) * P]
                )
                dlt = dnp.tile([OUT_F, nb * P], f32, tag="dl")
                nc.sync.dma_start(dlt[:], dinvl[:, b0 * P : (b0 + nb) * P])
                accs = []
                for _bl in range(nb):
                    acc_t = psl2.tile([OUT_F, P], f32, tag="acc2")
                    accs.append(acc_t)
                # last (k, j) per block for stop flags
                last_kj = {}
                for bl in range(nb):
                    lk = None
                    for k in range(NBUCKET):
                        if cap2_ch[b0 + bl][k] > 0:
                            lk = (k, cap2_ch[b0 + bl][k] - 1)
                    last_kj[bl] = lk
                for bl in range(nb):
                    selft = blk.tile([P, OUT_F], bf, tag="self")
                    nc.sync.dma_start(
                        selft[:],
                        h2_local[(b0 + bl) * P : (b0 + bl + 1) * P, 0:OUT_F],
                    )
                    nc.tensor.matmul(
                        accs[bl][:], lhsT=selft[:], rhs=idt[:],
                        start=True, stop=(last_kj[bl] is None),
                    )
                for k in range(NBUCKET):
                    gt2 = gts[k]
                    blbase = 0
                    for bl in range(nb):
                        nchb = cap2_ch[b0 + bl][k]
                        for j in range(nchb):
                            c = blbase + j
                            cg = (kbase[k] - chunk_base) + c
                            nc.tensor.matmul(
                                accs[bl][:],
                                lhsT=gt2[:, c * P : c * P + OUT_F],
                                rhs=oht[:, cg * P : (cg + 1) * P],
                                start=False,
                                stop=(last_kj[bl] == (k, j)),
                            )
                        blbase += nchb
                for bl in range(nb):
                    tmp = blk.tile([OUT_F, P], f32, tag="tmp")
                    nc.vector.tensor_tensor(
                        out=tmp[:], in0=accs[bl][:],
                        in1=dlt[:, bl * P : (bl + 1) * P], op=mult,
                    )
                    outt = blk.tile([OUT_F, P], f32, tag="outt")
                    nc.vector.tensor_scalar(
                        out=outt[:], in0=tmp[:], scalar1=b2t[:, :1],
                        scalar2=None, op0=add,
                    )
                    nc.sync.dma_start(
                        outT[:, (b0 + bl) * P : (b0 + bl + 1) * P], outt[:]
                    )
                chunk_base = cb

    nc.compile()
    _BUILD_CACHE[key] = nc
    return nc


def _run(inputs, trace=False):
    from concourse.bass_utils import run_bass_kernel_spmd

    key, per_core, consts, perm_pos = _prep(
        inputs["x"], inputs["edge_index"], inputs["W1"], inputs["b1"],
        inputs["W2"], inputs["b2"],
    )
    nc = _build(key)
    in_maps = [{**consts, **per_core[s]} for s in range(NCORES)]
    res = run_bass_kernel_spmd(
        nc, in_maps, core_ids=list(range(NCORES)), trace=trace
    )
    all_out = np.concatenate(
        [np.ascontiguousarray(res.results[s]["outT"].T) for s in range(NCORES)],
        axis=0,
    )
    out = np.ascontiguousarray(all_out[perm_pos])
    return out, res


def kernel(**inputs) -> np.ndarray:
    out, _ = _run(inputs, trace=False)
    return out


# revision 8
# speedup vs baseline: 1.0749x; 1.0749x over previous
"""Trainium2 Bass kernel for a 2-layer GCN encoder (N=100000, E=1600000, 128->128->64).

v2 strategy (8 NeuronCores, SPMD):
  out = A_hat @ relu(A_hat @ X @ W1 + b1) @ W2 + b2,  A_hat = D^-1/2 (A+I) D^-1/2

  Nodes are degree-sorted and dealt into 784 blocks of 128 (block g ->
  core g%8, local index g//8), so the 8 cores' bl-th blocks have nearly
  identical degree profiles and one static program serves all cores with
  per-block chunk counts.

  Layer 1 (zero one-hots): the per-edge source rows of x are host-gathered,
  norm-folded ((dinv_u*dinv_v) folded in, self-loops included) and stored
  FEAT-MAJOR bf16 in identity-routing layout: chunk j, lane d = j-th edge
  of the dest in slot d (zero rows pad). Then t1[hid,dest] accumulates
  with ONE weight-stationary matmul per chunk: t1 += W1^T @ chunkT.
  Tail: h1 = relu(t1+b1) (ACT, bf16), h2p[dest,o] = h1-lhsT @ W2,
  h2s = h2p * dinv_dest (ACT per-partition scale), duplicated to 128 bf16
  cols so the L2 gather element is 256B.

  AllGather h2_local bf16 [12544,128] -> h2_full [100352,128].

  Layer 2: self-loop contributions via contiguous dma_start from the
  core's OWN h2_local rows + identity matmul (no gather descriptors).
  Non-self edges: SWDGE dma_gather (int16 idx, 4 position buckets) of
  256B bf16 rows, routed lane->dest by bf16 is_eq one-hots (160ns vs
  785ns for the baseline's f32 eq+mult): acc[o,dest] += g[:, :64]^T @ st.
  Tail: DVE mult by dinv_dest tile + add b2; output written transposed
  [64, rows]; host un-transposes and un-permutes.
"""

import math

import numpy as np
import ml_dtypes

N = 100000
E = 1600000
IN_F = 128
HID = 128
OUT_F = 64
NCORES = 8
P = 128
BLOCKS_PER_CORE = 98
NBLOCKS = NCORES * BLOCKS_PER_CORE  # 784
ROWS_PER_CORE = BLOCKS_PER_CORE * P  # 12544
NBUCKET = 4
L2_BUCKET_ROWS = 25088
L1_GROUP_CH = 72   # max chunks per L1 stream group
L2_GROUP_CH = 24   # max chunks per (bucket-call) in an L2 group

_BUILD_CACHE = {}

bf16 = ml_dtypes.bfloat16


def _ranks(key, ncells):
    order = np.argsort(key, kind="stable")
    key_sorted = key[order]
    counts = np.bincount(key_sorted, minlength=ncells)
    starts = np.zeros_like(counts)
    starts[1:] = np.cumsum(counts)[:-1]
    rank_sorted = np.arange(order.size, dtype=np.int64) - starts[key_sorted]
    rank = np.empty(order.size, dtype=np.int64)
    rank[order] = rank_sorted
    return rank, counts


def _pack_gidx(idx_stream):
    m = idx_stream.reshape(-1, 16).T
    return np.ascontiguousarray(np.tile(m, (8, 1)))


def _l1_groups(p1_list):
    """Group consecutive blocks while total chunks <= L1_GROUP_CH."""
    out = []
    b0 = 0
    while b0 < BLOCKS_PER_CORE:
        nb = 0
        tot = 0
        while b0 + nb < BLOCKS_PER_CORE and (
            nb == 0 or tot + p1_list[b0 + nb] <= L1_GROUP_CH
        ):
            tot += p1_list[b0 + nb]
            nb += 1
        out.append((b0, nb))
        b0 += nb
    return out


def _l2_groups(cap2):
    """Group consecutive blocks while per-bucket chunk total <= L2_GROUP_CH."""
    out = []
    b0 = 0
    while b0 < BLOCKS_PER_CORE:
        nb = 0
        while b0 + nb < BLOCKS_PER_CORE and nb < 4:
            cand = [
                sum(cap2[b0 + i][k] for i in range(nb + 1))
                for k in range(NBUCKET)
            ]
            if nb > 0 and max(cand) > L2_GROUP_CH:
                break
            nb += 1
        out.append((b0, nb))
        b0 += nb
    return out


def _prep(x, edge_index, W1, b1, W2, b2):
    x = np.ascontiguousarray(np.asarray(x, dtype=np.float32))
    ei = np.asarray(edge_index, dtype=np.int64)
    loops = np.arange(N, dtype=np.int64)
    row1 = np.concatenate([ei[0], loops])   # L1 includes self loops
    col1 = np.concatenate([ei[1], loops])

    degi = np.bincount(col1, minlength=N)   # includes self loop
    dinv = (1.0 / np.sqrt(degi.astype(np.float64))).astype(np.float32)
    norm1 = dinv[row1] * dinv[col1]

    # degree-sorted deal: sorted rank r -> global block r//P -> core g%8, bl g//8
    order = np.argsort(-degi, kind="stable")
    gblock = np.empty(N, np.int64)
    slot_of = np.empty(N, np.int64)
    r = np.arange(N, dtype=np.int64)
    gblock[order] = r // P
    slot_of[order] = r % P
    core_of = gblock % NCORES
    bl_of = gblock // NCORES
    perm_pos = core_of * ROWS_PER_CORE + bl_of * P + slot_of

    deg_sorted = degi[order]
    # p1_list[bl] = max degree among the 8 cores' bl-th blocks
    p1_list = [int(deg_sorted[bl * NCORES * P]) for bl in range(BLOCKS_PER_CORE)]
    l1_off = np.zeros(BLOCKS_PER_CORE + 1, np.int64)
    l1_off[1:] = np.cumsum(p1_list)
    tot1 = int(l1_off[-1]) * P  # per-core L1 edge slots

    # L1 positions: per core, block bl region at l1_off[bl]*P; chunk j lane d
    rankd, _ = _ranks(col1, N)
    c1 = col1
    pos1 = core_of[c1] * tot1 + (l1_off[bl_of[c1]] + rankd) * P + slot_of[c1]
    src1 = np.zeros(NCORES * tot1, np.int64)
    n1 = np.zeros(NCORES * tot1, np.float32)
    src1[pos1] = row1
    n1[pos1] = norm1

    # ---- layer 2 (non-self edges): 4 position buckets, one-hot routing ----
    dcore = core_of[ei[1]]
    dbl = bl_of[ei[1]]
    dloc_all = slot_of[ei[1]].astype(np.float32)
    cpos = perm_pos[ei[0]]
    b2k = cpos // L2_BUCKET_ROWS
    i2 = (cpos - b2k * L2_BUCKET_ROWS).astype(np.int16)
    key2 = (dcore * BLOCKS_PER_CORE + dbl) * NBUCKET + b2k
    rank2, cnt2 = _ranks(key2, NBLOCKS * NBUCKET)
    cnt2m = cnt2.reshape(NCORES, BLOCKS_PER_CORE, NBUCKET)
    cap2 = [
        [
            int(math.ceil(int(cnt2m[:, bl, k].max()) / P)) * P
            for k in range(NBUCKET)
        ]
        for bl in range(BLOCKS_PER_CORE)
    ]  # slots per (bl, bucket)

    groups2 = _l2_groups([[c // P for c in row] for row in cap2])
    # stream order: group g -> bucket k -> block bl -> slots
    cell_start = np.zeros((BLOCKS_PER_CORE, NBUCKET), np.int64)
    off = 0
    for b0, nb in groups2:
        for k in range(NBUCKET):
            for bl in range(nb):
                cell_start[b0 + bl, k] = off
                off += cap2[b0 + bl][k]
    tot2 = off  # per-core L2 edge slots

    pos2 = dcore * tot2 + cell_start[dbl, b2k] + rank2
    i2s = np.zeros(NCORES * tot2, np.int16)
    i2s[pos2] = i2
    ohfull = np.zeros((NCORES * tot2, P), bf16)
    ohfull[pos2, slot_of[ei[1]]] = bf16(1.0)

    dinv_posarr = np.zeros(NBLOCKS * P, np.float32)
    dinv_posarr[perm_pos] = dinv

    per_core = []
    for s in range(NCORES):
        sl1 = slice(s * tot1, (s + 1) * tot1)
        sl2 = slice(s * tot2, (s + 1) * tot2)
        xs = x[src1[sl1]] * n1[sl1][:, None]  # [tot1, IN_F] f32
        nch1 = tot1 // P
        xgT = np.ascontiguousarray(
            xs.reshape(nch1, P, IN_F).transpose(2, 0, 1).reshape(IN_F, -1)
        ).astype(bf16)
        dslice = dinv_posarr[s * ROWS_PER_CORE : (s + 1) * ROWS_PER_CORE]
        per_core.append(
            {
                "xgT": xgT,
                "dinvp": np.ascontiguousarray(dslice.reshape(-1, P).T),
                "dinvl": np.ascontiguousarray(np.tile(dslice, (OUT_F, 1))),
                "gidx2": _pack_gidx(i2s[sl2]),
                "oh": np.ascontiguousarray(
                    ohfull[sl2].reshape(-1, P, P).transpose(1, 0, 2).reshape(P, -1)
                ),
            }
        )

    iota_np = np.tile(np.arange(P, dtype=np.float32), (P, 1))
    consts = {
        "w1": np.ascontiguousarray(np.asarray(W1, np.float32)).astype(bf16),
        "w2": np.ascontiguousarray(np.asarray(W2, np.float32)).astype(bf16),
        "b1": np.ascontiguousarray(np.asarray(b1, np.float32).reshape(HID, 1)),
        "b2": np.ascontiguousarray(np.asarray(b2, np.float32).reshape(OUT_F, 1)),
        "iotab": np.ascontiguousarray(iota_np).astype(bf16),
        "identb": np.eye(P, dtype=np.float32).astype(bf16),
    }
    key = (tuple(p1_list), tuple(tuple(r_) for r_ in cap2))
    return key, per_core, consts, perm_pos


def _build(key):
    if key in _BUILD_CACHE:
        return _BUILD_CACHE[key]

    import concourse.bass as bass  # noqa: F401
    import concourse.bacc as bacc
    import concourse.mybir as mybir
    import concourse.tile as tile

    p1_list, cap2 = key
    p1_list = list(p1_list)
    cap2_ch = [[c // P for c in row] for row in cap2]
    f32 = mybir.dt.float32
    bf = mybir.dt.bfloat16
    i16 = mybir.dt.int16
    groups1 = _l1_groups(p1_list)
    groups2 = _l2_groups(cap2_ch)
    l1_off = [0]
    for p in p1_list:
        l1_off.append(l1_off[-1] + p)
    nch1 = l1_off[-1]
    nch2 = sum(
        cap2_ch[b0 + bl][k]
        for b0, nb in groups2
        for k in range(NBUCKET)
        for bl in range(nb)
    )

    nc = bacc.Bacc(
        "TRN2", target_bir_lowering=False, debug=False, num_devices=NCORES
    )
    xgT = nc.dram_tensor("xgT", [P, nch1 * P], bf, kind="ExternalInput")
    w1 = nc.dram_tensor("w1", [IN_F, HID], bf, kind="ExternalInput")
    w2 = nc.dram_tensor("w2", [HID, OUT_F], bf, kind="ExternalInput")
    b1 = nc.dram_tensor("b1", [HID, 1], f32, kind="ExternalInput")
    b2 = nc.dram_tensor("b2", [OUT_F, 1], f32, kind="ExternalInput")
    iotab = nc.dram_tensor("iotab", [P, P], bf, kind="ExternalInput")
    identb = nc.dram_tensor("identb", [P, P], bf, kind="ExternalInput")
    dinvp = nc.dram_tensor("dinvp", [P, BLOCKS_PER_CORE], f32, kind="ExternalInput")
    dinvl = nc.dram_tensor("dinvl", [OUT_F, ROWS_PER_CORE], f32, kind="ExternalInput")
    gidx2 = nc.dram_tensor("gidx2", [P, nch2 * P // 16], i16, kind="ExternalInput")
    oh = nc.dram_tensor("oh", [P, nch2 * P], bf, kind="ExternalInput")
    outT = nc.dram_tensor(
        "outT", [OUT_F, ROWS_PER_CORE], f32, kind="ExternalOutput"
    )

    relu = mybir.ActivationFunctionType.Relu
    copyf = mybir.ActivationFunctionType.Copy
    is_eq = mybir.AluOpType.is_equal
    mult = mybir.AluOpType.mult
    add = mybir.AluOpType.add

    with tile.TileContext(nc) as tc:
        with (
            tc.tile_pool(name="consts", bufs=1) as cp,
            tc.tile_pool(name="gat", bufs=2) as gat,
            tc.tile_pool(name="idxp", bufs=4) as idxp,
            tc.tile_pool(name="dnp", bufs=3) as dnp,
            tc.tile_pool(name="sp", bufs=6) as sp,
            tc.tile_pool(name="blk", bufs=6) as blk,
            tc.tile_pool(name="ps1", bufs=2, space="PSUM") as ps1,
            tc.tile_pool(name="ps2", bufs=2, space="PSUM") as ps2,
            tc.tile_pool(name="psl2", bufs=4, space="PSUM") as psl2,
            tc.tile_pool(name="dram", bufs=1, space="DRAM") as dram,
        ):
            w1t = cp.tile([IN_F, HID], bf)
            w2t = cp.tile([HID, OUT_F], bf)
            b1t = cp.tile([HID, 1], f32)
            b2t = cp.tile([OUT_F, 1], f32)
            iot = cp.tile([P, P], bf)
            idt = cp.tile([P, P], bf)
            dvp = cp.tile([P, BLOCKS_PER_CORE], f32)
            nc.sync.dma_start(w1t[:], w1[:])
            nc.sync.dma_start(w2t[:], w2[:])
            nc.sync.dma_start(b1t[:], b1[:])
            nc.sync.dma_start(b2t[:], b2[:])
            nc.sync.dma_start(iot[:], iotab[:])
            nc.sync.dma_start(idt[:], identb[:])
            nc.sync.dma_start(dvp[:], dinvp[:])

            h2_local = dram.tile([ROWS_PER_CORE, P], bf, tag="h2l")
            h2_full = dram.tile(
                [NCORES * ROWS_PER_CORE, P], bf, tag="h2f", addr_space="Shared"
            )

            # ---------------- Layer 1 ----------------
            for b0, nb in groups1:
                c0 = l1_off[b0]
                C = l1_off[b0 + nb] - c0
                gt = gat.tile([P, C * P], bf, tag="g")
                nc.sync.dma_start(gt[:], xgT[:, c0 * P : (c0 + C) * P])
                for bl in range(nb):
                    bb = b0 + bl
                    p1b = p1_list[bb]
                    cb = l1_off[bb] - c0
                    acc = ps1.tile([HID, P], f32, tag="acc1")
                    for j in range(p1b):
                        c = cb + j
                        nc.tensor.matmul(
                            acc[:],
                            lhsT=w1t[:],
                            rhs=gt[:, c * P : (c + 1) * P],
                            start=(j == 0),
                            stop=(j == p1b - 1),
                        )
                    h1 = blk.tile([HID, P], bf, tag="h1")
                    nc.scalar.activation(h1[:], acc[:], relu, bias=b1t[:, :1])
                    h2p = ps2.tile([P, OUT_F], f32, tag="h2p")
                    nc.tensor.matmul(
                        h2p[:], lhsT=h1[:], rhs=w2t[:], start=True, stop=True
                    )
                    h2s = blk.tile([P, P], bf, tag="h2s")
                    nc.scalar.activation(
                        h2s[:, 0:OUT_F], h2p[:], copyf, scale=dvp[:, bb : bb + 1]
                    )
                    nc.vector.tensor_copy(h2s[:, OUT_F:P], h2s[:, 0:OUT_F])
                    nc.sync.dma_start(h2_local[bb * P : (bb + 1) * P, :], h2s[:])

            # ---------------- AllGather ----------------
            nc.gpsimd.collective_compute(
                "AllGather",
                mybir.AluOpType.bypass,
                replica_groups=[list(range(NCORES))],
                ins=[h2_local.opt()],
                outs=[h2_full.opt()],
            )

            # ---------------- Layer 2 ----------------
            chunk_base = 0
            for b0, nb in groups2:
                kch = [
                    sum(cap2_ch[b0 + bl][k] for bl in range(nb))
                    for k in range(NBUCKET)
                ]
                gts = []
                kbase = []
                cb = chunk_base
                for k in range(NBUCKET):
                    nidx = kch[k] * P
                    gt2 = gat.tile([P, kch[k] * P], bf, tag=f"g2{k}")
                    it = idxp.tile([P, nidx // 16], i16, tag=f"i{k}")
                    nc.sync.dma_start(
                        it[:], gidx2[:, cb * P // 16 : (cb + kch[k]) * P // 16]
                    )
                    nc.gpsimd.dma_gather(
                        out_ap=gt2[:].rearrange("p (c e) -> p c e", e=P),
                        in_ap=h2_full[
                            k * L2_BUCKET_ROWS : (k + 1) * L2_BUCKET_ROWS, :
                        ],
                        idxs_ap=it[:],
                        num_idxs=nidx,
                        num_idxs_reg=nidx,
                        elem_size=P,
                        single_packet=False,
                    )
                    gts.append(gt2)
                    kbase.append(cb)
                    cb += kch[k]
                tot_ch = cb - chunk_base
                oht = dnp.tile([P, tot_ch * P], bf, tag="oh")
                nc.sync.dma_start(
                    oht[:], oh[:, chunk_base * P : (chunk_base + tot_ch) * P]
                )
                dlt = dnp.tile([OUT_F, nb * P], f32, tag="dl")
                nc.sync.dma_start(dlt[:], dinvl[:, b0 * P : (b0 + nb) * P])
                accs = []
                for _bl in range(nb):
                    acc_t = psl2.tile([OUT_F, P], f32, tag="acc2")
                    accs.append(acc_t)
                # last (k, j) per block for stop flags
                last_kj = {}
                for bl in range(nb):
                    lk = None
                    for k in range(NBUCKET):
                        if cap2_ch[b0 + bl][k] > 0:
                            lk = (k, cap2_ch[b0 + bl][k] - 1)
                    last_kj[bl] = lk
                for bl in range(nb):
                    selft = blk.tile([P, OUT_F], bf, tag="self")
                    nc.sync.dma_start(
                        selft[:],
                        h2_local[(b0 + bl) * P : (b0 + bl + 1) * P, 0:OUT_F],
                    )
                    nc.tensor.matmul(
                        accs[bl][:], lhsT=selft[:], rhs=idt[:],
                        start=True, stop=(last_kj[bl] is None),
                    )
                for k in range(NBUCKET):
                    gt2 = gts[k]
                    blbase = 0
                    for bl in range(nb):
                        nchb = cap2_ch[b0 + bl][k]
                        for j in range(nchb):
                            c = blbase + j
                            cg = (kbase[k] - chunk_base) + c
                            nc.tensor.matmul(
                                accs[bl][:],
                                lhsT=gt2[:, c * P : c * P + OUT_F],
                                rhs=oht[:, cg * P : (cg + 1) * P],
                                start=False,
                                stop=(last_kj[bl] == (k, j)),
                            )
                        blbase += nchb
                for bl in range(nb):
                    tmp = blk.tile([OUT_F, P], f32, tag="tmp")
                    nc.vector.tensor_tensor(
                        out=tmp[:], in0=accs[bl][:],
                        in1=dlt[:, bl * P : (bl + 1) * P], op=mult,
                    )
                    outt = blk.tile([OUT_F, P], f32, tag="outt")
                    nc.vector.tensor_scalar(
                        out=outt[:], in0=tmp[:], scalar1=b2t[:, :1],
                        scalar2=None, op0=add,
                    )
                    nc.sync.dma_start(
                        outT[:, (b0 + bl) * P : (b0 + bl + 1) * P], outt[:]
                    )
                chunk_base = cb

    nc.compile()
    _BUILD_CACHE[key] = nc
    return nc


def _run(inputs, trace=False):
    from concourse.bass_utils import run_bass_kernel_spmd

    key, per_core, consts, perm_pos = _prep(
        inputs["x"], inputs["edge_index"], inputs["W1"], inputs["b1"],
        inputs["W2"], inputs["b2"],
    )
    nc = _build(key)
    in_maps = [{**consts, **per_core[s]} for s in range(NCORES)]
    res = run_bass_kernel_spmd(
        nc, in_maps, core_ids=list(range(NCORES)), trace=trace
    )
    all_out = np.concatenate(
        [np.ascontiguousarray(res.results[s]["outT"].T) for s in range(NCORES)],
        axis=0,
    )
    out = np.ascontiguousarray(all_out[perm_pos])
    return out, res


def kernel(**inputs) -> np.ndarray:
    out, _ = _run(inputs, trace=False)
    return out


# revision 11
# speedup vs baseline: 1.0832x; 1.0077x over previous
"""Trainium2 Bass kernel for a 2-layer GCN encoder (N=100000, E=1600000, 128->128->64).

v2 strategy (8 NeuronCores, SPMD):
  out = A_hat @ relu(A_hat @ X @ W1 + b1) @ W2 + b2,  A_hat = D^-1/2 (A+I) D^-1/2

  Nodes are degree-sorted and dealt into 784 blocks of 128 (block g ->
  core g%8, local index g//8), so the 8 cores' bl-th blocks have nearly
  identical degree profiles and one static program serves all cores with
  per-block chunk counts.

  Layer 1 (zero one-hots): the per-edge source rows of x are host-gathered,
  norm-folded ((dinv_u*dinv_v) folded in, self-loops included) and stored
  FEAT-MAJOR bf16 in identity-routing layout: chunk j, lane d = j-th edge
  of the dest in slot d (zero rows pad). Then t1[hid,dest] accumulates
  with ONE weight-stationary matmul per chunk: t1 += W1^T @ chunkT.
  Tail: h1 = relu(t1+b1) (ACT, bf16), h2p[dest,o] = h1-lhsT @ W2,
  h2s = h2p * dinv_dest (ACT per-partition scale), duplicated to 128 bf16
  cols so the L2 gather element is 256B.

  AllGather h2_local bf16 [12544,128] -> h2_full [100352,128].

  Layer 2: self-loop contributions via contiguous dma_start from the
  core's OWN h2_local rows + identity matmul (no gather descriptors).
  Non-self edges: SWDGE dma_gather (int16 idx, 4 position buckets) of
  256B bf16 rows, routed lane->dest by bf16 is_eq one-hots (160ns vs
  785ns for the baseline's f32 eq+mult): acc[o,dest] += g[:, :64]^T @ st.
  Tail: DVE mult by dinv_dest tile + add b2; output written transposed
  [64, rows]; host un-transposes and un-permutes.
"""

import math

import numpy as np
import ml_dtypes

N = 100000
E = 1600000
IN_F = 128
HID = 128
OUT_F = 64
NCORES = 8
P = 128
BLOCKS_PER_CORE = 98
NBLOCKS = NCORES * BLOCKS_PER_CORE  # 784
ROWS_PER_CORE = BLOCKS_PER_CORE * P  # 12544
NBUCKET = 4
L2_BUCKET_ROWS = 25088
L1_GROUP_CH = 72   # max chunks per L1 stream group
L2_GROUP_CH = 24   # max chunks per (bucket-call) in an L2 group

_BUILD_CACHE = {}

bf16 = ml_dtypes.bfloat16


def _ranks(key, ncells):
    order = np.argsort(key, kind="stable")
    key_sorted = key[order]
    counts = np.bincount(key_sorted, minlength=ncells)
    starts = np.zeros_like(counts)
    starts[1:] = np.cumsum(counts)[:-1]
    rank_sorted = np.arange(order.size, dtype=np.int64) - starts[key_sorted]
    rank = np.empty(order.size, dtype=np.int64)
    rank[order] = rank_sorted
    return rank, counts


def _pack_gidx(idx_stream):
    m = idx_stream.reshape(-1, 16).T
    return np.ascontiguousarray(np.tile(m, (8, 1)))


def _l1_groups(p1_list):
    """Group consecutive blocks while total chunks <= L1_GROUP_CH."""
    out = []
    b0 = 0
    while b0 < BLOCKS_PER_CORE:
        nb = 0
        tot = 0
        while b0 + nb < BLOCKS_PER_CORE and (
            nb == 0 or tot + p1_list[b0 + nb] <= L1_GROUP_CH
        ):
            tot += p1_list[b0 + nb]
            nb += 1
        out.append((b0, nb))
        b0 += nb
    return out


def _l2_groups(cap2):
    """Group consecutive blocks while per-bucket chunk total <= L2_GROUP_CH."""
    out = []
    b0 = 0
    while b0 < BLOCKS_PER_CORE:
        nb = 0
        while b0 + nb < BLOCKS_PER_CORE and nb < 4:
            cand = [
                sum(cap2[b0 + i][k] for i in range(nb + 1))
                for k in range(NBUCKET)
            ]
            if nb > 0 and max(cand) > L2_GROUP_CH:
                break
            nb += 1
        out.append((b0, nb))
        b0 += nb
    return out


def _prep(x, edge_index, W1, b1, W2, b2):
    x = np.ascontiguousarray(np.asarray(x, dtype=np.float32))
    ei = np.asarray(edge_index, dtype=np.int64)
    loops = np.arange(N, dtype=np.int64)
    row1 = np.concatenate([ei[0], loops])   # L1 includes self loops
    col1 = np.concatenate([ei[1], loops])

    degi = np.bincount(col1, minlength=N)   # includes self loop
    dinv = (1.0 / np.sqrt(degi.astype(np.float64))).astype(np.float32)
    norm1 = dinv[row1] * dinv[col1]

    # degree-sorted deal: sorted rank r -> global block r//P -> core g%8, bl g//8
    order = np.argsort(-degi, kind="stable")
    gblock = np.empty(N, np.int64)
    slot_of = np.empty(N, np.int64)
    r = np.arange(N, dtype=np.int64)
    gblock[order] = r // P
    slot_of[order] = r % P
    core_of = gblock % NCORES
    bl_of = gblock // NCORES
    perm_pos = core_of * ROWS_PER_CORE + bl_of * P + slot_of

    deg_sorted = degi[order]
    # p1_list[bl] = max degree among the 8 cores' bl-th blocks
    p1_list = [int(deg_sorted[bl * NCORES * P]) for bl in range(BLOCKS_PER_CORE)]
    l1_off = np.zeros(BLOCKS_PER_CORE + 1, np.int64)
    l1_off[1:] = np.cumsum(p1_list)
    tot1 = int(l1_off[-1]) * P  # per-core L1 edge slots

    # L1 positions: per core, block bl region at l1_off[bl]*P; chunk j lane d
    rankd, _ = _ranks(col1, N)
    c1 = col1
    pos1 = core_of[c1] * tot1 + (l1_off[bl_of[c1]] + rankd) * P + slot_of[c1]
    src1 = np.zeros(NCORES * tot1, np.int64)
    n1 = np.zeros(NCORES * tot1, np.float32)
    src1[pos1] = row1
    n1[pos1] = norm1

    # ---- layer 2 (non-self edges): 4 position buckets, one-hot routing ----
    dcore = core_of[ei[1]]
    dbl = bl_of[ei[1]]
    dloc_all = slot_of[ei[1]].astype(np.float32)
    cpos = perm_pos[ei[0]]
    b2k = cpos // L2_BUCKET_ROWS
    i2 = (cpos - b2k * L2_BUCKET_ROWS).astype(np.int16)
    key2 = (dcore * BLOCKS_PER_CORE + dbl) * NBUCKET + b2k
    rank2, cnt2 = _ranks(key2, NBLOCKS * NBUCKET)
    cnt2m = cnt2.reshape(NCORES, BLOCKS_PER_CORE, NBUCKET)
    cap2 = [
        [
            int(math.ceil(int(cnt2m[:, bl, k].max()) / P)) * P
            for k in range(NBUCKET)
        ]
        for bl in range(BLOCKS_PER_CORE)
    ]  # slots per (bl, bucket)

    groups2 = _l2_groups([[c // P for c in row] for row in cap2])
    # stream order: group g -> bucket k -> block bl -> slots
    cell_start = np.zeros((BLOCKS_PER_CORE, NBUCKET), np.int64)
    off = 0
    for b0, nb in groups2:
        for k in range(NBUCKET):
            for bl in range(nb):
                cell_start[b0 + bl, k] = off
                off += cap2[b0 + bl][k]
    tot2 = off  # per-core L2 edge slots

    pos2 = dcore * tot2 + cell_start[dbl, b2k] + rank2
    i2s = np.zeros(NCORES * tot2, np.int16)
    i2s[pos2] = i2
    ohfull = np.zeros((NCORES * tot2, P), bf16)
    ohfull[pos2, slot_of[ei[1]]] = bf16(1.0)

    dinv_posarr = np.zeros(NBLOCKS * P, np.float32)
    dinv_posarr[perm_pos] = dinv

    per_core = []
    for s in range(NCORES):
        sl1 = slice(s * tot1, (s + 1) * tot1)
        sl2 = slice(s * tot2, (s + 1) * tot2)
        xs = x[src1[sl1]] * n1[sl1][:, None]  # [tot1, IN_F] f32
        nch1 = tot1 // P
        xgT = np.ascontiguousarray(
            xs.reshape(nch1, P, IN_F).transpose(2, 0, 1).reshape(IN_F, -1)
        ).astype(bf16)
        dslice = dinv_posarr[s * ROWS_PER_CORE : (s + 1) * ROWS_PER_CORE]
        per_core.append(
            {
                "xgT": xgT,
                "dinvp": np.ascontiguousarray(dslice.reshape(-1, P).T),
                "dinvl": np.ascontiguousarray(np.tile(dslice, (OUT_F, 1))),
                "gidx2": _pack_gidx(i2s[sl2]),
                "oh": np.ascontiguousarray(
                    ohfull[sl2].reshape(-1, P, P).transpose(1, 0, 2).reshape(P, -1)
                ),
            }
        )

    iota_np = np.tile(np.arange(P, dtype=np.float32), (P, 1))
    consts = {
        "w1": np.ascontiguousarray(np.asarray(W1, np.float32)).astype(bf16),
        "w2": np.ascontiguousarray(np.asarray(W2, np.float32)).astype(bf16),
        "b1": np.ascontiguousarray(np.asarray(b1, np.float32).reshape(HID, 1)),
        "b2": np.ascontiguousarray(np.asarray(b2, np.float32).reshape(OUT_F, 1)),
        "iotab": np.ascontiguousarray(iota_np).astype(bf16),
        "identb": np.eye(P, dtype=np.float32).astype(bf16),
    }
    key = (tuple(p1_list), tuple(tuple(r_) for r_ in cap2))
    return key, per_core, consts, perm_pos


def _build(key):
    if key in _BUILD_CACHE:
        return _BUILD_CACHE[key]

    import concourse.bass as bass  # noqa: F401
    import concourse.bacc as bacc
    import concourse.mybir as mybir
    import concourse.tile as tile

    p1_list, cap2 = key
    p1_list = list(p1_list)
    cap2_ch = [[c // P for c in row] for row in cap2]
    f32 = mybir.dt.float32
    bf = mybir.dt.bfloat16
    i16 = mybir.dt.int16
    groups1 = _l1_groups(p1_list)
    groups2 = _l2_groups(cap2_ch)
    l1_off = [0]
    for p in p1_list:
        l1_off.append(l1_off[-1] + p)
    nch1 = l1_off[-1]
    nch2 = sum(
        cap2_ch[b0 + bl][k]
        for b0, nb in groups2
        for k in range(NBUCKET)
        for bl in range(nb)
    )

    nc = bacc.Bacc(
        "TRN2", target_bir_lowering=False, debug=False, num_devices=NCORES
    )
    xgT = nc.dram_tensor("xgT", [P, nch1 * P], bf, kind="ExternalInput")
    w1 = nc.dram_tensor("w1", [IN_F, HID], bf, kind="ExternalInput")
    w2 = nc.dram_tensor("w2", [HID, OUT_F], bf, kind="ExternalInput")
    b1 = nc.dram_tensor("b1", [HID, 1], f32, kind="ExternalInput")
    b2 = nc.dram_tensor("b2", [OUT_F, 1], f32, kind="ExternalInput")
    iotab = nc.dram_tensor("iotab", [P, P], bf, kind="ExternalInput")
    identb = nc.dram_tensor("identb", [P, P], bf, kind="ExternalInput")
    dinvp = nc.dram_tensor("dinvp", [P, BLOCKS_PER_CORE], f32, kind="ExternalInput")
    dinvl = nc.dram_tensor("dinvl", [OUT_F, ROWS_PER_CORE], f32, kind="ExternalInput")
    gidx2 = nc.dram_tensor("gidx2", [P, nch2 * P // 16], i16, kind="ExternalInput")
    oh = nc.dram_tensor("oh", [P, nch2 * P], bf, kind="ExternalInput")
    outT = nc.dram_tensor(
        "outT", [OUT_F, ROWS_PER_CORE], f32, kind="ExternalOutput"
    )

    relu = mybir.ActivationFunctionType.Relu
    copyf = mybir.ActivationFunctionType.Copy
    is_eq = mybir.AluOpType.is_equal
    mult = mybir.AluOpType.mult
    add = mybir.AluOpType.add

    with tile.TileContext(nc) as tc:
        with (
            tc.tile_pool(name="consts", bufs=1) as cp,
            tc.tile_pool(name="gat1", bufs=3) as gat1,
            tc.tile_pool(name="gat", bufs=2) as gat,
            tc.tile_pool(name="idxp", bufs=4) as idxp,
            tc.tile_pool(name="dnp", bufs=3) as dnp,
            tc.tile_pool(name="sp", bufs=6) as sp,
            tc.tile_pool(name="blk", bufs=6) as blk,
            tc.tile_pool(name="ps1", bufs=2, space="PSUM") as ps1,
            tc.tile_pool(name="ps2", bufs=2, space="PSUM") as ps2,
            tc.tile_pool(name="psl2", bufs=4, space="PSUM") as psl2,
            tc.tile_pool(name="dram", bufs=1, space="DRAM") as dram,
        ):
            w1t = cp.tile([IN_F, HID], bf)
            w2t = cp.tile([HID, OUT_F], bf)
            b1t = cp.tile([HID, 1], f32)
            b2t = cp.tile([OUT_F, 1], f32)
            iot = cp.tile([P, P], bf)
            idt = cp.tile([P, P], bf)
            dvp = cp.tile([P, BLOCKS_PER_CORE], f32)
            nc.sync.dma_start(w1t[:], w1[:])
            nc.sync.dma_start(w2t[:], w2[:])
            nc.sync.dma_start(b1t[:], b1[:])
            nc.sync.dma_start(b2t[:], b2[:])
            nc.sync.dma_start(iot[:], iotab[:])
            nc.sync.dma_start(idt[:], identb[:])
            nc.sync.dma_start(dvp[:], dinvp[:])

            h2_local = dram.tile([ROWS_PER_CORE, P], bf, tag="h2l")
            h2_full = dram.tile(
                [NCORES * ROWS_PER_CORE, P], bf, tag="h2f", addr_space="Shared"
            )

            # ---------------- Layer 1 ----------------
            for b0, nb in groups1:
                c0 = l1_off[b0]
                C = l1_off[b0 + nb] - c0
                gt = gat1.tile([P, C * P], bf, tag="g")
                nc.sync.dma_start(gt[:], xgT[:, c0 * P : (c0 + C) * P])
                for bl in range(nb):
                    bb = b0 + bl
                    p1b = p1_list[bb]
                    cb = l1_off[bb] - c0
                    acc = ps1.tile([HID, P], f32, tag="acc1")
                    for j in range(p1b):
                        c = cb + j
                        nc.tensor.matmul(
                            acc[:],
                            lhsT=w1t[:],
                            rhs=gt[:, c * P : (c + 1) * P],
                            start=(j == 0),
                            stop=(j == p1b - 1),
                        )
                    h1 = blk.tile([HID, P], bf, tag="h1")
                    nc.scalar.activation(h1[:], acc[:], relu, bias=b1t[:, :1])
                    h2p = ps2.tile([P, OUT_F], f32, tag="h2p")
                    nc.tensor.matmul(
                        h2p[:], lhsT=h1[:], rhs=w2t[:], start=True, stop=True
                    )
                    h2s = blk.tile([P, P], bf, tag="h2s")
                    nc.scalar.activation(
                        h2s[:, 0:OUT_F], h2p[:], copyf, scale=dvp[:, bb : bb + 1]
                    )
                    nc.vector.tensor_copy(h2s[:, OUT_F:P], h2s[:, 0:OUT_F])
                    nc.sync.dma_start(h2_local[bb * P : (bb + 1) * P, :], h2s[:])

            # ---------------- AllGather ----------------
            nc.gpsimd.collective_compute(
                "AllGather",
                mybir.AluOpType.bypass,
                replica_groups=[list(range(NCORES))],
                ins=[h2_local.opt()],
                outs=[h2_full.opt()],
            )

            # ---------------- Layer 2 ----------------
            chunk_base = 0
            for b0, nb in groups2:
                kch = [
                    sum(cap2_ch[b0 + bl][k] for bl in range(nb))
                    for k in range(NBUCKET)
                ]
                gts = []
                kbase = []
                cb = chunk_base
                for k in range(NBUCKET):
                    nidx = kch[k] * P
                    gt2 = gat.tile([P, kch[k] * P], bf, tag=f"g2{k}")
                    it = idxp.tile([P, nidx // 16], i16, tag=f"i{k}")
                    nc.sync.dma_start(
                        it[:], gidx2[:, cb * P // 16 : (cb + kch[k]) * P // 16]
                    )
                    nc.gpsimd.dma_gather(
                        out_ap=gt2[:].rearrange("p (c e) -> p c e", e=P),
                        in_ap=h2_full[
                            k * L2_BUCKET_ROWS : (k + 1) * L2_BUCKET_ROWS, :
                        ],
                        idxs_ap=it[:],
                        num_idxs=nidx,
                        num_idxs_reg=nidx,
                        elem_size=P,
                        single_packet=False,
                    )
                    gts.append(gt2)
                    kbase.append(cb)
                    cb += kch[k]
                tot_ch = cb - chunk_base
                oht = dnp.tile([P, tot_ch * P], bf, tag="oh")
                nc.sync.dma_start(
                    oht[:], oh[:, chunk_base * P : (chunk_base + tot_ch) * P]
                )
                dlt = dnp.tile([OUT_F, nb * P], f32, tag="dl")
                nc.sync.dma_start(dlt[:], dinvl[:, b0 * P : (b0 + nb) * P])
                accs = []
                for _bl in range(nb):
                    acc_t = psl2.tile([OUT_F, P], f32, tag="acc2")
                    accs.append(acc_t)
                # last (k, j) per block for stop flags
                last_kj = {}
                for bl in range(nb):
                    lk = None
                    for k in range(NBUCKET):
                        if cap2_ch[b0 + bl][k] > 0:
                            lk = (k, cap2_ch[b0 + bl][k] - 1)
                    last_kj[bl] = lk
                for bl in range(nb):
                    selft = blk.tile([P, OUT_F], bf, tag="self")
                    nc.sync.dma_start(
                        selft[:],
                        h2_local[(b0 + bl) * P : (b0 + bl + 1) * P, 0:OUT_F],
                    )
                    nc.tensor.matmul(
                        accs[bl][:], lhsT=selft[:], rhs=idt[:],
                        start=True, stop=(last_kj[bl] is None),
                    )
                for k in range(NBUCKET):
                    gt2 = gts[k]
                    blbase = 0
                    for bl in range(nb):
                        nchb = cap2_ch[b0 + bl][k]
                        for j in range(nchb):
                            c = blbase + j
                            cg = (kbase[k] - chunk_base) + c
                            nc.tensor.matmul(
                                accs[bl][:],
                                lhsT=gt2[:, c * P : c * P + OUT_F],
                                rhs=oht[:, cg * P : (cg + 1) * P],
                                start=False,
                                stop=(last_kj[bl] == (k, j)),
                            )
                        blbase += nchb
                for bl in range(nb):
                    tmp = blk.tile([OUT_F, P], f32, tag="tmp")
                    nc.vector.tensor_tensor(
                        out=tmp[:], in0=accs[bl][:],
                        in1=dlt[:, bl * P : (bl + 1) * P], op=mult,
                    )
                    outt = blk.tile([OUT_F, P], f32, tag="outt")
                    nc.vector.tensor_scalar(
                        out=outt[:], in0=tmp[:], scalar1=b2t[:, :1],
                        scalar2=None, op0=add,
                    )
                    nc.sync.dma_start(
                        outT[:, (b0 + bl) * P : (b0 + bl + 1) * P], outt[:]
                    )
                chunk_base = cb

    nc.compile()
    _BUILD_CACHE[key] = nc
    return nc


def _run(inputs, trace=False):
    from concourse.bass_utils import run_bass_kernel_spmd

    key, per_core, consts, perm_pos = _prep(
        inputs["x"], inputs["edge_index"], inputs["W1"], inputs["b1"],
        inputs["W2"], inputs["b2"],
    )
    nc = _build(key)
    in_maps = [{**consts, **per_core[s]} for s in range(NCORES)]
    res = run_bass_kernel_spmd(
        nc, in_maps, core_ids=list(range(NCORES)), trace=trace
    )
    all_out = np.concatenate(
        [np.ascontiguousarray(res.results[s]["outT"].T) for s in range(NCORES)],
        axis=0,
    )
    out = np.ascontiguousarray(all_out[perm_pos])
    return out, res


def kernel(**inputs) -> np.ndarray:
    out, _ = _run(inputs, trace=False)
    return out
